# revision 5
# baseline (speedup 1.0000x reference)
"""Trainium2 Bass kernel for nn_Microscope (PSF scatter-add), v2.

Sharding: 8 cores = (b in 0..4) x (h-half in {0,1}); each core owns output
rows (b, half*128..+128).  Emitters whose patch rows intersect the slab are
processed there (boundary emitters duplicated; clipped rows route nowhere).

Key idea vs v1 (per-emitter matmuls, 441 PE cols/emitter): pack G=6 emitters
with CLOSE (w, d) into one group.  The host pre-aligns each emitter's 21^3
patch inside a shared footprint tile [126, Fw*Fd] (bf16, zeros elsewhere) at
its absolute (w, d) offset.  One matmul per (group x psum bank) then scatters
all 6 emitters at once: lhsT = Zg [126, 128] routes patch row (slot s, row k)
to output row h_e-10+k and multiplies by the emitter's normalization scale.
PE columns drop to footprint/G ~ 140/emitter.

Zg is built on-device per group in ONE GpSimd op from a host-shipped
per-partition shift vector:  Zg[p, r] = (iota[r] == shift[p]) * scale[p]
(tensor_scalar is_equal + mult).  shift = -1 encodes clipped/dummy rows.

Normalization: per-group row sums (engine rotates DVE reduce / ACT accum /
GpSimd accum), ind-matmul folds to per-emitter sums, reciprocal * i_val*1e6,
indT-matmul broadcasts scale back to the 126 partitions feeding Zg.

PSUM: ring of 7 banks, each [5 w x 96 d] f32 (d pitch holds d0 in [2,87));
bank 7 holds the normalization scratch.  Banks retire (ACT crop-copy ->
bf16 -> DMA out) as the w sweep advances.  Output assembled as f32 on host.
"""

import math
import threading
from contextlib import ExitStack

import ml_dtypes
import numpy as np

import concourse.bass as bass
import concourse.tile as tile
from concourse import bacc, mybir
from concourse import bass_utils

LAST = None
BF16 = mybir.dt.bfloat16
F32 = mybir.dt.float32
AF = mybir.ActivationFunctionType
ALU = mybir.AluOpType
BF16NP = ml_dtypes.bfloat16

BS, CH, H, W, D = 4, 1, 256, 256, 64
PH, PW, PD = 21, 21, 21
HALF = 128
SCALE_MULT = 10000.0 * 100.0   # folded into i_val

G = 6            # emitter slots per group (6*21 = 126 partitions)
GB = 8           # groups per normalization batch
RUN = G * GB     # emitters per w-run, re-sorted by d inside
PITCH = 96       # psum d pitch (holds pitch idx [2, 87))
BW = 5           # w columns per psum bank
NRING = 7        # ring banks (bank 7 = norm scratch)
WCAP = 31        # max footprint w extent (ring span <= 7 banks)
FCAP = 2048      # max footprint cols per partition (staging window size)
NW = 32          # staging windows
NBANKS_TOT = 56  # w idx [0, 276) / 5
# engine per group-slot for row sums: D=DVE reduce, A=ACT accum, P=GpSimd
SUMENG = "DADDADAD"


def _host_pack(psf16, i_val, b, h, w, d):
    """Per core: group emitters, build aligned tiles + shift/ival tables."""
    cores = []
    for core in range(8):
        b_t, half = core >> 1, core & 1
        lo = half * HALF
        sel = np.where((b == b_t) & (h - PH // 2 <= lo + HALF - 1)
                       & (h + PH // 2 >= lo))[0]
        if len(sel) == 0:
            cores.append(None)
            continue
        sel = sel[np.argsort(w[sel], kind="stable")]

        raw_groups = []
        for r0 in range(0, len(sel), RUN):
            run = sel[r0:r0 + RUN]
            run = run[np.argsort(d[run], kind="stable")]
            for c0 in range(0, len(run), G):
                raw_groups.append(run[c0:c0 + G])

        final = []

        def add_group(idx):
            ws, ds = w[idx], d[idx]
            fw = ws.max() - ws.min() + PW
            fd = ds.max() - ds.min() + PD
            if len(idx) > 1 and (fw > WCAP or fw * fd > FCAP):
                key = ws if fw > WCAP else ds
                order = idx[np.argsort(key, kind="stable")]
                m = len(idx) // 2
                add_group(order[:m])
                add_group(order[m:])
            else:
                final.append(idx)

        for idx in raw_groups:
            add_group(idx)
        final.sort(key=lambda idx: w[idx].min())

        ng = len(final)
        nbatch = (ng + GB - 1) // GB
        Fs, ginfo = [], []
        for idx in final:
            fw = w[idx].max() - w[idx].min() + PW
            fd = d[idx].max() - d[idx].min() + PD
            Fs.append(fw * fd)
        offs = np.concatenate([[0], np.cumsum(Fs)])
        sumf = int(offs[-1])

        psf_pack = np.zeros((126, sumf), BF16NP)
        shifts = np.full((126, ng), -1.0, np.float32)
        ivals = np.zeros((G, nbatch * GB), np.float32)
        kk = np.arange(PH)
        for gi, idx in enumerate(final):
            w_lo = int(w[idx].min()) - PW // 2
            dmin = int(d[idx].min())
            fw = int(w[idx].max()) - PW // 2 - w_lo + PW
            fd = int(d[idx].max()) - dmin + PD
            off = int(offs[gi])
            tile3 = np.zeros((126, fw, fd), BF16NP)
            for s, e in enumerate(idx):
                wr = int(w[e]) - PW // 2 - w_lo
                dr = int(d[e]) - dmin
                tile3[21 * s:21 * s + 21, wr:wr + PW, dr:dr + PD] = psf16[e]
                hv = int(h[e]) - PH // 2 + kk - lo
                shifts[21 * s:21 * s + 21, gi] = np.where(
                    (hv >= 0) & (hv < HALF), hv.astype(np.float32), -1.0)
                ivals[s, gi] = i_val[e] * SCALE_MULT
            for s in range(len(idx), G):
                tile3[21 * s, 0, 0] = 1.0   # sum guard for dummy slot
            psf_pack[:, off:off + fw * fd] = tile3.reshape(126, fw * fd)
            wi_lo = w_lo + PW // 2 + 10 - 10  # = w_lo + 10 in w_idx space
            wi_lo = w_lo + 10
            ginfo.append(dict(off=off, F=fw * fd, Fw=fw, Fd=fd,
                              wi_lo=wi_lo, p0=dmin + 2,
                              Tmin=wi_lo // BW, Tmax=(wi_lo + fw - 1) // BW))
        cores.append(dict(ne=len(sel), ng=ng, nbatch=nbatch, sumf=sumf,
                          psf=psf_pack, shifts=shifts, ival=ivals,
                          ginfo=ginfo))
    return cores


def _consts():
    iota = np.tile(np.arange(128, dtype=np.float32), (128, 1))    # [128,128]
    p = np.arange(126)
    ind = (p[:, None] // 21 == np.arange(G)[None, :]).astype(np.float32)
    indT = np.ascontiguousarray(ind.T)                            # [6,126]
    return iota, ind, indT


def _build_program(cd):
    ng, nbatch, sumf = cd["ng"], cd["nbatch"], cd["sumf"]
    ginfo = cd["ginfo"]
    nc = bacc.Bacc("TRN2", target_bir_lowering=False, debug=False)
    psf_d = nc.dram_tensor("psf", [126, sumf], BF16, kind="ExternalInput").ap()
    shifts_d = nc.dram_tensor("shifts", [126, ng], F32,
                              kind="ExternalInput").ap()
    ival_d = nc.dram_tensor("ival", [G, nbatch * GB], F32,
                            kind="ExternalInput").ap()
    iota_d = nc.dram_tensor("iota", [128, 128], F32, kind="ExternalInput").ap()
    ind_d = nc.dram_tensor("ind", [126, G], F32, kind="ExternalInput").ap()
    indT_d = nc.dram_tensor("indT", [G, 126], F32, kind="ExternalInput").ap()
    out_d = nc.dram_tensor("out", [HALF, W, D], BF16, kind="ExternalOutput").ap()

    with tile.TileContext(nc) as tc:
        with ExitStack() as ctx:
            const = ctx.enter_context(tc.tile_pool(name="const", bufs=1))
            evp = ctx.enter_context(tc.tile_pool(name="evp", bufs=4))
            psum = ctx.enter_context(tc.tile_pool(name="psum", bufs=1,
                                                  space="PSUM"))

            iota_t = const.tile([128, 128], F32)
            nc.gpsimd.dma_start(iota_t[:], iota_d[:])
            ind_t = const.tile([126, G], F32)
            nc.gpsimd.dma_start(ind_t[:], ind_d[:])
            indT_t = const.tile([G, 126], F32)
            nc.gpsimd.dma_start(indT_t[:], indT_d[:])
            shifts_t = const.tile([126, ng], F32)
            nc.gpsimd.dma_start(shifts_t[:], shifts_d[:])
            ival_t = const.tile([G, nbatch * GB], F32)
            nc.gpsimd.dma_start(ival_t[:], ival_d[:])

            stg = const.tile([128, NW * FCAP], BF16)
            zg_t = [const.tile([128, 128], BF16, tag=f"zg{i}", name=f"zg{i}")
                    for i in range(2)]
            rows_t = [const.tile([126, GB], F32, tag=f"rows{i}",
                                 name=f"rows{i}") for i in range(2)]
            recip_t = [const.tile([G, GB], F32, tag=f"recip{i}",
                                  name=f"recip{i}") for i in range(2)]
            scale_t = [const.tile([126, GB], F32, tag=f"scale{i}",
                                  name=f"scale{i}") for i in range(2)]
            for i in range(2):
                nc.vector.memset(rows_t[i][:], 1.0)

            ring = psum.tile([128, 8 * 512], F32)

            def bank(T):
                r = T % NRING
                return ring[:, 512 * r:512 * r + BW * PITCH].rearrange(
                    "p (w d) -> p w d", d=PITCH)

            def ps_sum_ap(k):
                o = 3584 + 8 * (k % 2)
                return ring[0:G, o:o + GB]

            def ps_bcast_ap(k):
                o = 3584 + 32 + 8 * (k % 2)
                return ring[0:126, o:o + GB]

            def win(g):
                F = ginfo[g]["F"]
                c0 = FCAP * (g % NW)
                return stg[0:126, c0:c0 + F]

            next_dma = 0
            next_sum = 0
            opened = -1
            retired = -1

            def dma_group(g):
                nonlocal next_dma
                gi = ginfo[g]
                nc.sync.dma_start(win(g), psf_d[:, gi["off"]:gi["off"] + gi["F"]])
                next_dma += 1

            def sum_group(g):
                nonlocal next_sum
                wv = win(g)
                col = rows_t[(g // GB) % 2][:, (g % GB):(g % GB) + 1]
                eng = SUMENG[g % len(SUMENG)]
                if eng == "D":
                    nc.vector.tensor_reduce(col, wv, mybir.AxisListType.X,
                                            ALU.add)
                else:
                    nc.scalar.activation(wv, wv, AF.Relu, accum_out=col)
                next_sum += 1

            def norm_batch(k):
                ps = ps_sum_ap(k)
                nc.tensor.matmul(ps, ind_t[:], rows_t[k % 2][:],
                                 start=True, stop=True, skip_group_check=True)
                rc = recip_t[k % 2]
                nc.vector.reciprocal(rc[:], ps)
                nc.vector.tensor_mul(rc[:], rc[:],
                                     ival_t[:, k * GB:k * GB + GB])
                pb = ps_bcast_ap(k)
                nc.tensor.matmul(pb, indT_t[:], rc[:],
                                 start=True, stop=True, skip_group_check=True)
                nc.vector.tensor_copy(scale_t[k % 2][:], pb)

            def retire(T):
                nonlocal retired
                retired = T
                if not (2 <= T <= 53):
                    return
                nw = BW if T < 53 else 1
                ev = evp.tile([128, BW * D], BF16, tag="ev", name="ev")
                ev3 = ev[:, 0:nw * D].rearrange("p (w d) -> p w d", d=D)
                nc.scalar.activation(ev3, bank(T)[:, 0:nw, 12:76], AF.Copy)
                wb = BW * T - 10
                nc.scalar.dma_start(out_d[:, wb:wb + nw, :], ev3)

            def open_through(T_hi):
                nonlocal opened
                while opened < T_hi:
                    opened += 1
                    if opened - NRING >= 0:
                        retire(opened - NRING)
                    r = opened % NRING
                    nc.vector.memset(ring[:, 512 * r:512 * r + BW * PITCH], 0.0)

            def zg_group(g):
                zg = zg_t[g % 2]
                nc.gpsimd.tensor_scalar(
                    zg[0:126, :], iota_t[0:126, :],
                    shifts_t[:, g:g + 1],
                    scale_t[(g // GB) % 2][:, (g % GB):(g % GB) + 1],
                    ALU.is_equal, ALU.mult)

            def emit_group(g):
                gi = ginfo[g]
                open_through(gi["Tmax"])
                zg = zg_t[g % 2]
                w3 = win(g).rearrange("p (w d) -> p w d", d=gi["Fd"])
                for T in range(gi["Tmin"], gi["Tmax"] + 1):
                    jb0 = max(0, BW * T - gi["wi_lo"])
                    jb1 = min(gi["Fw"], BW * T + BW - gi["wi_lo"])
                    wi0 = gi["wi_lo"] + jb0 - BW * T
                    out = bank(T)[:, wi0:wi0 + (jb1 - jb0),
                                  gi["p0"]:gi["p0"] + gi["Fd"]]
                    nc.tensor.matmul(out, zg[0:126, :], w3[:, jb0:jb1, :],
                                     start=False, stop=False,
                                     skip_group_check=True)

            # ---- schedule ----
            while next_dma < min(ng, 2 * GB):
                dma_group(next_dma)
            while next_sum < min(ng, GB):
                sum_group(next_sum)
            norm_batch(0)
            for g in range(ng):
                k = g // GB
                if g % GB == 0 and k + 1 < nbatch:
                    while next_dma < min(ng, (k + 3) * GB):
                        dma_group(next_dma)
                    while next_sum < min(ng, (k + 2) * GB):
                        sum_group(next_sum)
                    norm_batch(k + 1)
                zg_group(g)
                emit_group(g)
            for T in range(retired + 1, min(opened + 1, 54)):
                retire(T)

    nc.compile()
    return nc


def kernel(psf_raw, i_val, b, c, h, w, d):
    psf_raw = np.asarray(psf_raw)
    i_val = np.asarray(i_val)
    b = np.asarray(b); h = np.asarray(h); w = np.asarray(w); d = np.asarray(d)
    n = psf_raw.shape[0]
    psf16 = psf_raw.reshape(n, PH, PW, PD).astype(BF16NP)

    cores = _host_pack(psf16, i_val, b, h, w, d)
    iota, ind, indT = _consts()

    ncs = [None] * 8
    errs = []

    def build(i):
        try:
            if cores[i] is not None:
                ncs[i] = _build_program(cores[i])
        except BaseException as exc:
            errs.append((i, exc))
            raise

    threads = [threading.Thread(target=build, args=(i,)) for i in range(8)]
    for t in threads:
        t.start()
    for t in threads:
        t.join()
    if errs:
        raise errs[0][1]

    import jax
    devices = jax.devices()
    results = [None] * 8

    def run(i):
        if ncs[i] is None:
            results[i] = {"out": np.zeros((HALF, W, D), BF16NP)}
            return
        cd = cores[i]
        in_map = {
            "psf": cd["psf"], "shifts": cd["shifts"], "ival": cd["ival"],
            "iota": iota, "ind": ind, "indT": indT,
        }
        try:
            with jax.default_device(devices[i]):
                res = bass_utils.run_bass_kernel_spmd(ncs[i], [in_map],
                                                      core_ids=[0])
            results[i] = res.results[0]
        except BaseException as exc:
            errs.append((i, exc))
            raise

    rthreads = [threading.Thread(target=run, args=(i,)) for i in range(8)]
    for t in rthreads:
        t.start()
    for t in rthreads:
        t.join()
    if errs:
        raise errs[0][1]

    global LAST
    LAST = {"cores": cores, "ncs": ncs, "iota": iota, "ind": ind, "indT": indT}

    out = np.zeros((BS, CH, H, W, D), np.float32)
    for core in range(8):
        b_t, half = core >> 1, core & 1
        out[b_t, 0, half * HALF:(half + 1) * HALF] = \
            np.asarray(results[core]["out"]).astype(np.float32)
    return out
